# revision 21
# baseline (speedup 1.0000x reference)
"""Trainium2 Bass kernel for nn_DecoderFactoredLSTM (v5).

Factored-LSTM decoder:
  emb = B_w[captions]                       [B,T,E] -> tokens [T*B, E]
  u   = emb @ (V^T S^T U^T) + bias          [T*B, 4H]   (gate pre-activations)
  recurrence over T=40 steps (LSTM, no tanh on c for h)
  out = hiddens @ C_w^T + C_b               [T*B, V]

v5 strategy:
  * Host folds the whole input path into one fused table
        BM = B_w @ (U S V)^T + gate_bias        [V, 4H] bf16
    (rows interleaved by h-half) so the device gathers gate
    pre-activations u directly, one 128-partition indirect DMA/step.
  * Recurrence in bf16 with 2-way column-tiled matmuls: each [K=128,
    M=64, N=512] pair runs concurrently on PE column groups 0-1 / 2-3
    (psum partitions 0:64 / 64:128), full-array efficiency at batch 64.
    Gate/psum layout [128=(half,b), 512] keeps every element-wise op
    and activation full-width.
  * h transposed for the next step via DMA xbar transpose (SBUF->SBUF,
    off the PE), directly into the layout both consumers need.
  * Vocab projection (8-way vocab-sharded, bf16) interleaved lag-1-pair
    to fill the PE during the recurrence chain.

Layouts:
  psum tile g: partition (half*64 + b), col j  ->  gate g, h = 512*half + j
  u table BMab[2v+half, g*512+j] = BM[v, g*1024 + 512*half + j]
  hTr slot: [128=j, 4=cc, 2=parity, 128=(half,b)], h = 512*half + 128*cc + j
"""

import sys

if "/opt/trn_rl_repo" not in sys.path:
    sys.path.insert(0, "/opt/trn_rl_repo")

import ml_dtypes
import numpy as np

import concourse.bass as bass
import concourse.mybir as mybir
import concourse.tile as tile
from concourse import bacc
from concourse.bass import ts
from concourse.bass_utils import run_bass_kernel_spmd
from concourse.masks import make_identity

B, T, E, H, F, V = 64, 40, 512, 1024, 512, 32000
NCORES = 8
VS = V // NCORES  # vocab slice per core: 4000
TOK = T * B  # 2560 tokens
NV = VS // 8  # 500 vocab cols per chunk
F32 = mybir.dt.float32
BF16 = mybir.dt.bfloat16
I32 = mybir.dt.int32
SIG = mybir.ActivationFunctionType.Sigmoid
TANH = mybir.ActivationFunctionType.Tanh


def _build():
    nc = bacc.Bacc(None, target_bir_lowering=False, debug=False)

    with tile.TileContext(nc) as tc:
        idx_d = nc.declare_dram_parameter("idx", [128, T], I32, isOutput=False)
        # row 2v+half = BM[v, half-block]; idx pre-doubled on host so one
        # 128-partition gather fetches both halves (partitions 64+ get +1)
        BM_d = nc.declare_dram_parameter("BMab", [2 * V, 2048], BF16, isOutput=False)
        W_d = nc.declare_dram_parameter("Wg", [128, 8, 4 * H], BF16, isOutput=False)
        CT_d = nc.declare_dram_parameter("CTc", [128, 8, 8, NV], BF16, isOutput=False)
        Cb_d = nc.declare_dram_parameter("Cb", [128, VS], BF16, isOutput=False)

        out_d = nc.declare_dram_parameter("out", [TOK, VS], F32, isOutput=True)

        with (
            tc.tile_pool(name="const", bufs=1) as const,
            tc.tile_pool(name="pers", bufs=1) as pB,
            tc.tile_pool(name="psg", bufs=1, space="PSUM") as pBg,
            tc.tile_pool(name="psv", bufs=1, space="PSUM") as pBv,
        ):
            id128b = const.tile([128, 128], BF16, tag="id128b")
            make_identity(nc, id128b)

            idx_s = pB.tile([128, T], I32, tag="idx")
            nc.sync.dma_start(idx_s[:], idx_d[:])

            # force both ACT tables (sigmoid, tanh) to load before the
            # scalar queue fills with weight-DMA issues
            warm = pB.tile([128, 1], F32, tag="warm")
            nc.scalar.activation(warm[:], id128b[:, 0:1], SIG)
            nc.scalar.activation(warm[:], id128b[:, 0:1], TANH)

            # u tiles: ring of 4, [128=(half,b), 2048=(g,j)] bf16
            ut = [pB.tile([128, 2048], BF16, tag=f"ut{s}", name=f"ut{s}") for s in range(4)]

            def gather_u(t):
                s = t % 4
                nc.gpsimd.indirect_dma_start(
                    out=ut[s][:],
                    out_offset=None,
                    in_=BM_d[:],
                    in_offset=bass.IndirectOffsetOnAxis(ap=idx_s[:, t : t + 1], axis=0),
                )

            for t in range(3):
                gather_u(t)

            # weights (loaded while steps 0-1 run; emitted after step 0 below)
            W_s = pB.tile([128, 8, 4 * H], BF16, tag="wg")
            CT16 = pB.tile([128, 8, 8, NV], BF16, tag="ct16")
            Cb16 = pB.tile([128, VS], BF16, tag="cb16")

            def load_weights():
                for k in range(4):
                    nc.sync.dma_start(W_s[:, k, :], W_d[:, k, :])
                for k in range(4, 8):
                    nc.scalar.dma_start(W_s[:, k, :], W_d[:, k, :])
                for n in range(4):
                    nc.sync.dma_start(CT16[:, n], CT_d[:, n])
                for n in range(4, 8):
                    nc.scalar.dma_start(CT16[:, n], CT_d[:, n])
                nc.scalar.dma_start(Cb16[:], Cb_d[:])

            # state
            # hT ring: [128=j, 4=cc, 2=half, 2=parity, 64=b] bf16
            hTr = [
                pB.tile([128, 4, 2, 2, 64], BF16, tag=f"hT{s}", name=f"hT{s}")
                for s in range(3)
            ]
            gs = pB.tile([128, 2048], BF16, tag="gs")
            sig = pB.tile([128, 1536], BF16, tag="sig")
            th = pB.tile([128, 512], BF16, tag="th")
            tmp1 = pB.tile([128, 512], F32, tag="tmp1")
            h16 = pB.tile([128, 512], BF16, tag="h16")
            cst = [pB.tile([128, 512], F32, tag=f"cs{s}", name=f"cs{s}") for s in range(2)]

            def vocab_chunks(m, chunks):
                hp = hTr[m % 3]
                for n in chunks:
                    pv = pBv.tile([128, NV], F32, tag=f"v{n % 3}", name=f"v{n % 3}")
                    for k in range(8):
                        nc.tensor.matmul(
                            pv[:],
                            lhsT=hp[:, k % 4, k // 4, :, :],
                            rhs=CT16[:, n, k, :],
                            start=(k == 0),
                            stop=(k == 7),
                        )
                    pev = pB.tile([128, NV], F32, tag=f"pe{n % 2}", name=f"pe{n % 2}")
                    nc.vector.tensor_add(pev[:], pv[:], Cb16[:, ts(n, NV)])
                    nc.sync.dma_start(out_d[ts(m, 128), ts(n, NV)], pev[:])

            def step(t):
                parity = t % 2
                slot = (t // 2) % 3
                utile = ut[t % 4]
                vm = (t - 2) // 2 if t >= 2 else None
                voff = 4 * parity

                cn = cst[t % 2]
                cp = cst[1 - t % 2]
                if t == 0:
                    nc.scalar.activation(sig[:, :], utile[:, :1536], SIG)
                    nc.scalar.activation(th[:], utile[:, 1536:], TANH)
                    nc.vector.tensor_mul(cn[:], sig[:, 0:512], th[:])
                    nc.vector.tensor_mul(h16[:], sig[:, 1024:1536], cn[:])
                else:
                    hprev = hTr[((t - 1) // 2) % 3]
                    pparity = (t - 1) % 2
                    gp = [
                        pBg.tile([128, 512], F32, tag=f"gp{g}", name=f"gp{g}")
                        for g in range(4)
                    ]
                    # c~ first so tanh + i*c~ start early; then i, f, o
                    for g in (3, 0, 1, 2):
                        for k in range(8):
                            hk = hprev[:, k % 4, k // 4, pparity, :]
                            # two concurrent col-group MMs (psum partitions
                            # 0:64 / 64:128); the sim's zero-region group
                            # check is partition-base-unaware, skip it
                            nc.tensor.matmul(
                                gp[g][0:64, :],
                                lhsT=hk,
                                rhs=W_s[:, k, g * 1024 : g * 1024 + 512],
                                start=(k == 0),
                                stop=(k == 7),
                                skip_group_check=True,
                            )
                            nc.tensor.matmul(
                                gp[g][64:128, :],
                                lhsT=hk,
                                rhs=W_s[:, k, g * 1024 + 512 : (g + 1) * 1024],
                                start=(k == 0),
                                stop=(k == 7),
                                skip_group_check=True,
                            )
                        # gs = psum + u  (bf16 out)
                        nc.vector.tensor_add(
                            gs[:, ts(g, 512)], gp[g][:], utile[:, ts(g, 512)]
                        )
                        if g == 3:
                            nc.scalar.activation(th[:], gs[:, ts(g, 512)], TANH)
                        else:
                            nc.scalar.activation(
                                sig[:, ts(g, 512)], gs[:, ts(g, 512)], SIG
                            )
                        if g == 0:
                            # e = i * c~ on gpsimd, off the DVE critical path
                            nc.gpsimd.tensor_mul(tmp1[:], sig[:, 0:512], th[:])
                        elif g == 1:
                            # d = f * c_prev, then c_new = d + e
                            nc.vector.tensor_mul(cn[:], sig[:, 512:1024], cp[:])
                            nc.vector.tensor_add(cn[:], cn[:], tmp1[:])
                        elif g == 2:
                            nc.vector.tensor_mul(h16[:], sig[:, 1024:1536], cn[:])

                # prefetch u for step t+3 (after the gpsimd chain op is queued)
                if t + 3 < T:
                    gather_u(t + 3)

                # h^T via DMA xbar transposes (off the PE), straight into the
                # layout the gate/vocab matmuls consume; halves split across
                # the two HWDGE queues
                for cc in range(4):
                    for half in range(2):
                        eng = nc.sync if half == 0 else nc.scalar
                        eng.dma_start(
                            hTr[slot][:, cc, half, parity, :],
                            h16[64 * half : 64 * half + 64, ts(cc, 128)],
                            transpose=True,
                        )

                if vm is not None:
                    vocab_chunks(vm, range(voff, voff + 4))

            step(0)
            load_weights()
            for t in range(1, T):
                step(t)
            vocab_chunks(19, range(8))

    nc.compile()
    return nc


def kernel(**inputs):
    captions = np.asarray(inputs["captions"])
    B_w = np.asarray(inputs["B_w"], dtype=np.float32)
    V_w = np.asarray(inputs["V_w"], dtype=np.float32)
    V_b = np.asarray(inputs["V_b"], dtype=np.float32)
    S_w = np.asarray(inputs["S_w"], dtype=np.float32)
    S_b = np.asarray(inputs["S_b"], dtype=np.float32)
    U_w = np.asarray(inputs["U_w"], dtype=np.float32)
    U_b = np.asarray(inputs["U_b"], dtype=np.float32)
    W_w = np.asarray(inputs["W_w"], dtype=np.float32)
    W_b = np.asarray(inputs["W_b"], dtype=np.float32)
    C_w = np.asarray(inputs["C_w"], dtype=np.float32)
    C_b = np.asarray(inputs["C_b"], dtype=np.float32)

    bf16 = ml_dtypes.bfloat16

    # --- host-side weight prep ---
    # fused input path: BM = B_w @ (U S V)^T + gate_bias   [V, 4H]
    M2 = np.stack([U_w[g] @ S_w[g] @ V_w[g] for g in range(4)])  # [4, H, E]
    bs = np.einsum("gf,gof->go", V_b, S_w) + S_b
    bu = np.einsum("gf,ghf->gh", bs, U_w) + U_b
    gate_bias = bu + W_b  # [4, H]
    BM = B_w @ M2.reshape(4 * H, E).T + gate_bias.reshape(1, 4 * H)  # [V, 4H]
    # [v, half, g*512+j] = BM[v, g*1024+512*half+j]; halves interleaved row-wise
    BMab = np.ascontiguousarray(
        BM.reshape(V, 4, 2, 512).transpose(0, 2, 1, 3).reshape(2 * V, 2048).astype(bf16)
    )

    # W, gate-major cols, k-chunked [ki, ko, 4H]
    Wgm = W_w.transpose(2, 0, 1).reshape(H, 4 * H)
    Wk = np.ascontiguousarray(
        Wgm.reshape(8, 128, 4 * H).transpose(1, 0, 2).astype(bf16)
    )

    # captions doubled: partition p fetches table row 2*cap + (p // 64)
    capBT = captions.astype(np.int32)  # [B, T]
    idx = np.ascontiguousarray(
        np.concatenate([2 * capBT, 2 * capBT + 1], axis=0)
    )  # [128, T]

    CT = C_w.T  # [H, V]

    nc = _build()

    in_maps = []
    for c in range(NCORES):
        CTc = CT[:, c * VS : (c + 1) * VS]
        # [ki, n, ko, j] = CT[ko*128+ki, n*500+j] — chunk-contiguous slices
        CTr = np.ascontiguousarray(
            CTc.reshape(8, 128, 8, NV).transpose(1, 2, 0, 3).astype(bf16)
        )
        in_maps.append(
            {
                "idx": idx,
                "BMab": BMab,
                "Wg": Wk,
                "CTc": CTr,
                "Cb": np.ascontiguousarray(
                    np.broadcast_to(C_b[c * VS : (c + 1) * VS], (128, VS)).astype(bf16)
                ),
            }
        )

    global _last_in_maps
    _last_in_maps = in_maps

    res = run_bass_kernel_spmd(nc, in_maps, list(range(NCORES)))
    out = np.concatenate([res.results[c]["out"] for c in range(NCORES)], axis=1)
    return out.astype(np.float32)


_last_in_maps = None


# revision 26
# speedup vs baseline: 1.3605x; 1.3605x over previous
"""Trainium2 Bass kernel for nn_DecoderFactoredLSTM (v5).

Factored-LSTM decoder:
  emb = B_w[captions]                       [B,T,E] -> tokens [T*B, E]
  u   = emb @ (V^T S^T U^T) + bias          [T*B, 4H]   (gate pre-activations)
  recurrence over T=40 steps (LSTM, no tanh on c for h)
  out = hiddens @ C_w^T + C_b               [T*B, V]

v5 strategy:
  * Host folds the whole input path into one fused table
        BM = B_w @ (U S V)^T + gate_bias        [V, 4H] bf16
    (rows interleaved by h-half) so the device gathers gate
    pre-activations u directly, one 128-partition indirect DMA/step.
  * Recurrence in bf16 with 2-way column-tiled matmuls: each [K=128,
    M=64, N=512] pair runs concurrently on PE column groups 0-1 / 2-3
    (psum partitions 0:64 / 64:128), full-array efficiency at batch 64.
    Gate/psum layout [128=(half,b), 512] keeps every element-wise op
    and activation full-width.
  * h transposed for the next step via DMA xbar transpose (SBUF->SBUF,
    off the PE), directly into the layout both consumers need.
  * Vocab projection (8-way vocab-sharded, bf16) interleaved lag-1-pair
    to fill the PE during the recurrence chain.

Layouts:
  psum tile g: partition (half*64 + b), col j  ->  gate g, h = 512*half + j
  u table BMab[2v+half, g*512+j] = BM[v, g*1024 + 512*half + j]
  hTr slot: [128=j, 4=cc, 2=parity, 128=(half,b)], h = 512*half + 128*cc + j
"""

import sys

if "/opt/trn_rl_repo" not in sys.path:
    sys.path.insert(0, "/opt/trn_rl_repo")

import ml_dtypes
import numpy as np

import concourse.bass as bass
import concourse.mybir as mybir
import concourse.tile as tile
from concourse import bacc
from concourse.bass import ts
from concourse.bass_utils import run_bass_kernel_spmd
from concourse.masks import make_identity

B, T, E, H, F, V = 64, 40, 512, 1024, 512, 32000
NCORES = 8
VS = V // NCORES  # vocab slice per core: 4000
TOK = T * B  # 2560 tokens
NV = VS // 8  # 500 vocab cols per chunk
F32 = mybir.dt.float32
BF16 = mybir.dt.bfloat16
I32 = mybir.dt.int32
SIG = mybir.ActivationFunctionType.Sigmoid
TANH = mybir.ActivationFunctionType.Tanh


def _build():
    nc = bacc.Bacc(None, target_bir_lowering=False, debug=False)

    with tile.TileContext(nc) as tc:
        idx_d = nc.declare_dram_parameter("idx", [128, T], I32, isOutput=False)
        # row 2v+half = BM[v, half-block]; idx pre-doubled on host so one
        # 128-partition gather fetches both halves (partitions 64+ get +1)
        BM_d = nc.declare_dram_parameter("BMab", [2 * V, 2048], BF16, isOutput=False)
        W_d = nc.declare_dram_parameter("Wg", [128, 8, 4 * H], BF16, isOutput=False)
        CT_d = nc.declare_dram_parameter("CTc", [128, 8, 8, NV], BF16, isOutput=False)
        Cb_d = nc.declare_dram_parameter("Cb", [128, VS], BF16, isOutput=False)

        out_d = nc.declare_dram_parameter("out", [TOK, VS], F32, isOutput=True)

        with (
            tc.tile_pool(name="const", bufs=1) as const,
            tc.tile_pool(name="pers", bufs=1) as pB,
            tc.tile_pool(name="psg", bufs=1, space="PSUM") as pBg,
            tc.tile_pool(name="psv", bufs=1, space="PSUM") as pBv,
            tc.tile_pool(name="pst", bufs=2, space="PSUM") as pBt,
        ):
            id128b = const.tile([128, 128], BF16, tag="id128b")
            make_identity(nc, id128b)

            idx_s = pB.tile([128, T], I32, tag="idx")
            nc.sync.dma_start(idx_s[:], idx_d[:])

            # force both ACT tables (sigmoid, tanh) to load before the
            # scalar queue fills with weight-DMA issues
            warm = pB.tile([128, 1], F32, tag="warm")
            nc.scalar.activation(warm[:], id128b[:, 0:1], SIG)
            nc.scalar.activation(warm[:], id128b[:, 0:1], TANH)

            # u tiles: ring of 4, [128=(half,b), 2048=(g,j)] bf16
            ut = [pB.tile([128, 2048], BF16, tag=f"ut{s}", name=f"ut{s}") for s in range(4)]

            def gather_u(t):
                s = t % 4
                nc.gpsimd.indirect_dma_start(
                    out=ut[s][:],
                    out_offset=None,
                    in_=BM_d[:],
                    in_offset=bass.IndirectOffsetOnAxis(ap=idx_s[:, t : t + 1], axis=0),
                )

            for t in range(3):
                gather_u(t)

            # weights (loaded while steps 0-1 run; emitted after step 0 below)
            W_s = pB.tile([128, 8, 4 * H], BF16, tag="wg")
            CT16 = pB.tile([128, 8, 8, NV], BF16, tag="ct16")
            Cb16 = pB.tile([128, VS], BF16, tag="cb16")

            def load_weights():
                for k in range(4):
                    nc.sync.dma_start(W_s[:, k, :], W_d[:, k, :])
                for k in range(4, 8):
                    nc.scalar.dma_start(W_s[:, k, :], W_d[:, k, :])
                for n in range(4):
                    nc.sync.dma_start(CT16[:, n], CT_d[:, n])
                for n in range(4, 8):
                    nc.scalar.dma_start(CT16[:, n], CT_d[:, n])
                nc.scalar.dma_start(Cb16[:], Cb_d[:])

            # state
            # hT ring: [128=j, 2=half, 4=cc, 2=parity, 64=b] bf16
            hTr = [
                pB.tile([128, 2, 4, 2, 64], BF16, tag=f"hT{s}", name=f"hT{s}")
                for s in range(3)
            ]
            gs = pB.tile([128, 2048], BF16, tag="gs")
            sig = pB.tile([128, 1536], BF16, tag="sig")
            th = pB.tile([128, 512], BF16, tag="th")
            tmp1 = pB.tile([128, 512], F32, tag="tmp1")
            h16 = pB.tile([128, 512], BF16, tag="h16")
            cst = [pB.tile([128, 512], F32, tag=f"cs{s}", name=f"cs{s}") for s in range(2)]

            def vocab_chunks(m, chunks):
                hp = hTr[m % 3]
                for n in chunks:
                    pv = pBv.tile([128, NV], F32, tag=f"v{n % 2}", name=f"v{n % 2}")
                    for k in range(8):
                        nc.tensor.matmul(
                            pv[:],
                            lhsT=hp[:, k // 4, k % 4, :, :],
                            rhs=CT16[:, n, k, :],
                            start=(k == 0),
                            stop=(k == 7),
                        )
                    pev = pB.tile([128, NV], F32, tag=f"pe{n % 2}", name=f"pe{n % 2}")
                    nc.vector.tensor_add(pev[:], pv[:], Cb16[:, ts(n, NV)])
                    nc.sync.dma_start(out_d[ts(m, 128), ts(n, NV)], pev[:])

            def step(t):
                parity = t % 2
                slot = (t // 2) % 3
                utile = ut[t % 4]
                vm = (t - 2) // 2 if t >= 2 else None
                voff = 4 * parity

                cn = cst[t % 2]
                cp = cst[1 - t % 2]
                if t == 0:
                    nc.scalar.activation(sig[:, :], utile[:, :1536], SIG)
                    nc.scalar.activation(th[:], utile[:, 1536:], TANH)
                    nc.vector.tensor_mul(cn[:], sig[:, 0:512], th[:])
                    nc.vector.tensor_mul(h16[:], sig[:, 1024:1536], cn[:])
                else:
                    hprev = hTr[((t - 1) // 2) % 3]
                    pparity = (t - 1) % 2
                    gp = [
                        pBg.tile([128, 512], F32, tag=f"gp{g}", name=f"gp{g}")
                        for g in range(4)
                    ]
                    # c~ first so tanh + i*c~ start early; then i, f, o
                    for g in (3, 0, 1, 2):
                        for k in range(8):
                            hk = hprev[:, k // 4, k % 4, pparity, :]
                            # two concurrent col-group MMs (psum partitions
                            # 0:64 / 64:128); the sim's zero-region group
                            # check is partition-base-unaware, skip it
                            nc.tensor.matmul(
                                gp[g][0:64, :],
                                lhsT=hk,
                                rhs=W_s[:, k, g * 1024 : g * 1024 + 512],
                                start=(k == 0),
                                stop=(k == 7),
                                skip_group_check=True,
                            )
                            nc.tensor.matmul(
                                gp[g][64:128, :],
                                lhsT=hk,
                                rhs=W_s[:, k, g * 1024 + 512 : (g + 1) * 1024],
                                start=(k == 0),
                                stop=(k == 7),
                                skip_group_check=True,
                            )
                        # gs = psum + u  (bf16 out)
                        nc.vector.tensor_add(
                            gs[:, ts(g, 512)], gp[g][:], utile[:, ts(g, 512)]
                        )
                        if g == 3:
                            nc.scalar.activation(th[:], gs[:, ts(g, 512)], TANH)
                        else:
                            nc.scalar.activation(
                                sig[:, ts(g, 512)], gs[:, ts(g, 512)], SIG
                            )
                        if g == 0:
                            # e = i * c~ on gpsimd, off the DVE critical path
                            nc.gpsimd.tensor_mul(tmp1[:], sig[:, 0:512], th[:])
                        elif g == 1:
                            # d = f * c_prev, then c_new = d + e
                            nc.vector.tensor_mul(cn[:], sig[:, 512:1024], cp[:])
                            nc.vector.tensor_add(cn[:], cn[:], tmp1[:])
                        elif g == 2:
                            nc.vector.tensor_mul(h16[:], sig[:, 1024:1536], cn[:])

                # prefetch u for step t+3 (after the gpsimd chain op is queued)
                if t + 3 < T:
                    gather_u(t + 3)

                # first half of this step's vocab tile while h resolves
                if vm is not None:
                    vocab_chunks(vm, range(voff, voff + 2))
                # transpose h into hT layout: 4x [128,128] PE transposes
                for cc in range(4):
                    tp = pBt.tile([128, 128], BF16, tag="htp")
                    nc.tensor.transpose(tp[:], h16[:, ts(cc, 128)], id128b[:])
                    nc.vector.tensor_copy(hTr[slot][:, :, cc, parity, :], tp[:])
                if vm is not None:
                    vocab_chunks(vm, range(voff + 2, voff + 4))

            step(0)
            load_weights()
            for t in range(1, T):
                step(t)
            vocab_chunks(19, range(8))

    nc.compile()
    return nc


def kernel(**inputs):
    captions = np.asarray(inputs["captions"])
    B_w = np.asarray(inputs["B_w"], dtype=np.float32)
    V_w = np.asarray(inputs["V_w"], dtype=np.float32)
    V_b = np.asarray(inputs["V_b"], dtype=np.float32)
    S_w = np.asarray(inputs["S_w"], dtype=np.float32)
    S_b = np.asarray(inputs["S_b"], dtype=np.float32)
    U_w = np.asarray(inputs["U_w"], dtype=np.float32)
    U_b = np.asarray(inputs["U_b"], dtype=np.float32)
    W_w = np.asarray(inputs["W_w"], dtype=np.float32)
    W_b = np.asarray(inputs["W_b"], dtype=np.float32)
    C_w = np.asarray(inputs["C_w"], dtype=np.float32)
    C_b = np.asarray(inputs["C_b"], dtype=np.float32)

    bf16 = ml_dtypes.bfloat16

    # --- host-side weight prep ---
    # fused input path: BM = B_w @ (U S V)^T + gate_bias   [V, 4H]
    M2 = np.stack([U_w[g] @ S_w[g] @ V_w[g] for g in range(4)])  # [4, H, E]
    bs = np.einsum("gf,gof->go", V_b, S_w) + S_b
    bu = np.einsum("gf,ghf->gh", bs, U_w) + U_b
    gate_bias = bu + W_b  # [4, H]
    BM = B_w @ M2.reshape(4 * H, E).T + gate_bias.reshape(1, 4 * H)  # [V, 4H]
    # [v, half, g*512+j] = BM[v, g*1024+512*half+j]; halves interleaved row-wise
    BMab = np.ascontiguousarray(
        BM.reshape(V, 4, 2, 512).transpose(0, 2, 1, 3).reshape(2 * V, 2048).astype(bf16)
    )

    # W, gate-major cols, k-chunked [ki, ko, 4H]
    Wgm = W_w.transpose(2, 0, 1).reshape(H, 4 * H)
    Wk = np.ascontiguousarray(
        Wgm.reshape(8, 128, 4 * H).transpose(1, 0, 2).astype(bf16)
    )

    # captions doubled: partition p fetches table row 2*cap + (p // 64)
    capBT = captions.astype(np.int32)  # [B, T]
    idx = np.ascontiguousarray(
        np.concatenate([2 * capBT, 2 * capBT + 1], axis=0)
    )  # [128, T]

    CT = C_w.T  # [H, V]

    nc = _build()

    in_maps = []
    for c in range(NCORES):
        CTc = CT[:, c * VS : (c + 1) * VS]
        # [ki, n, ko, j] = CT[ko*128+ki, n*500+j] — chunk-contiguous slices
        CTr = np.ascontiguousarray(
            CTc.reshape(8, 128, 8, NV).transpose(1, 2, 0, 3).astype(bf16)
        )
        in_maps.append(
            {
                "idx": idx,
                "BMab": BMab,
                "Wg": Wk,
                "CTc": CTr,
                "Cb": np.ascontiguousarray(
                    np.broadcast_to(C_b[c * VS : (c + 1) * VS], (128, VS)).astype(bf16)
                ),
            }
        )

    global _last_in_maps
    _last_in_maps = in_maps

    res = run_bass_kernel_spmd(nc, in_maps, list(range(NCORES)))
    out = np.concatenate([res.results[c]["out"] for c in range(NCORES)], axis=1)
    return out.astype(np.float32)


_last_in_maps = None


# revision 28
# speedup vs baseline: 1.3700x; 1.0070x over previous
"""Trainium2 Bass kernel for nn_DecoderFactoredLSTM (v5).

Factored-LSTM decoder:
  emb = B_w[captions]                       [B,T,E] -> tokens [T*B, E]
  u   = emb @ (V^T S^T U^T) + bias          [T*B, 4H]   (gate pre-activations)
  recurrence over T=40 steps (LSTM, no tanh on c for h)
  out = hiddens @ C_w^T + C_b               [T*B, V]

v5 strategy:
  * Host folds the whole input path into one fused table
        BM = B_w @ (U S V)^T + gate_bias        [V, 4H] bf16
    (rows interleaved by h-half) so the device gathers gate
    pre-activations u directly, one 128-partition indirect DMA/step.
  * Recurrence in bf16 with 2-way column-tiled matmuls: each [K=128,
    M=64, N=512] pair runs concurrently on PE column groups 0-1 / 2-3
    (psum partitions 0:64 / 64:128), full-array efficiency at batch 64.
    Gate/psum layout [128=(half,b), 512] keeps every element-wise op
    and activation full-width.
  * h transposed for the next step via DMA xbar transpose (SBUF->SBUF,
    off the PE), directly into the layout both consumers need.
  * Vocab projection (8-way vocab-sharded, bf16) interleaved lag-1-pair
    to fill the PE during the recurrence chain.

Layouts:
  psum tile g: partition (half*64 + b), col j  ->  gate g, h = 512*half + j
  u table BMab[2v+half, g*512+j] = BM[v, g*1024 + 512*half + j]
  hTr slot: [128=j, 4=cc, 2=parity, 128=(half,b)], h = 512*half + 128*cc + j
"""

import sys

if "/opt/trn_rl_repo" not in sys.path:
    sys.path.insert(0, "/opt/trn_rl_repo")

import ml_dtypes
import numpy as np

import concourse.bass as bass
import concourse.mybir as mybir
import concourse.tile as tile
from concourse import bacc
from concourse.bass import ts
from concourse.bass_utils import run_bass_kernel_spmd
from concourse.masks import make_identity

B, T, E, H, F, V = 64, 40, 512, 1024, 512, 32000
NCORES = 8
VS = V // NCORES  # vocab slice per core: 4000
TOK = T * B  # 2560 tokens
NV = VS // 8  # 500 vocab cols per chunk
F32 = mybir.dt.float32
BF16 = mybir.dt.bfloat16
I32 = mybir.dt.int32
SIG = mybir.ActivationFunctionType.Sigmoid
TANH = mybir.ActivationFunctionType.Tanh


def _build():
    nc = bacc.Bacc(None, target_bir_lowering=False, debug=False)

    with tile.TileContext(nc) as tc:
        idx_d = nc.declare_dram_parameter("idx", [128, T], I32, isOutput=False)
        # row 2v+half = BM[v, half-block]; idx pre-doubled on host so one
        # 128-partition gather fetches both halves (partitions 64+ get +1)
        BM_d = nc.declare_dram_parameter("BMab", [2 * V, 2048], BF16, isOutput=False)
        W_d = nc.declare_dram_parameter("Wg", [128, 8, 4 * H], BF16, isOutput=False)
        CT_d = nc.declare_dram_parameter("CTc", [128, 8, 8, NV], BF16, isOutput=False)
        Cb_d = nc.declare_dram_parameter("Cb", [128, VS], BF16, isOutput=False)

        out_d = nc.declare_dram_parameter("out", [TOK, VS], F32, isOutput=True)

        with (
            tc.tile_pool(name="const", bufs=1) as const,
            tc.tile_pool(name="pers", bufs=1) as pB,
            tc.tile_pool(name="psg", bufs=1, space="PSUM") as pBg,
            tc.tile_pool(name="psv", bufs=1, space="PSUM") as pBv,
            tc.tile_pool(name="pst", bufs=2, space="PSUM") as pBt,
        ):
            id128b = const.tile([128, 128], BF16, tag="id128b")
            make_identity(nc, id128b)

            idx_s = pB.tile([128, T], I32, tag="idx")
            nc.sync.dma_start(idx_s[:], idx_d[:])

            # force both ACT tables (sigmoid, tanh) to load before the
            # scalar queue fills with weight-DMA issues
            warm = pB.tile([128, 1], F32, tag="warm")
            nc.scalar.activation(warm[:], id128b[:, 0:1], SIG)
            nc.scalar.activation(warm[:], id128b[:, 0:1], TANH)

            # u tiles: ring of 4, [128=(half,b), 2048=(g,j)] bf16
            ut = [pB.tile([128, 2048], BF16, tag=f"ut{s}", name=f"ut{s}") for s in range(4)]

            def gather_u(t):
                s = t % 4
                nc.gpsimd.indirect_dma_start(
                    out=ut[s][:],
                    out_offset=None,
                    in_=BM_d[:],
                    in_offset=bass.IndirectOffsetOnAxis(ap=idx_s[:, t : t + 1], axis=0),
                )

            for t in range(3):
                gather_u(t)

            # weights (loaded while steps 0-1 run; emitted after step 0 below)
            W_s = pB.tile([128, 8, 4 * H], BF16, tag="wg")
            CT16 = pB.tile([128, 8, 8, NV], BF16, tag="ct16")
            Cb16 = pB.tile([128, VS], BF16, tag="cb16")

            def load_weights():
                for k in range(4):
                    nc.sync.dma_start(W_s[:, k, :], W_d[:, k, :])
                for k in range(4, 8):
                    nc.scalar.dma_start(W_s[:, k, :], W_d[:, k, :])

            def load_vocab_weights():
                for n in range(4):
                    nc.sync.dma_start(CT16[:, n], CT_d[:, n])
                for n in range(4, 8):
                    nc.scalar.dma_start(CT16[:, n], CT_d[:, n])
                nc.scalar.dma_start(Cb16[:], Cb_d[:])

            # state
            # hT ring: [128=j, 2=half, 4=cc, 2=parity, 64=b] bf16
            hTr = [
                pB.tile([128, 2, 4, 2, 64], BF16, tag=f"hT{s}", name=f"hT{s}")
                for s in range(3)
            ]
            gs = pB.tile([128, 2048], BF16, tag="gs")
            sig = pB.tile([128, 1536], BF16, tag="sig")
            th = pB.tile([128, 512], BF16, tag="th")
            tmp1 = pB.tile([128, 512], F32, tag="tmp1")
            h16 = pB.tile([128, 512], BF16, tag="h16")
            cst = [pB.tile([128, 512], F32, tag=f"cs{s}", name=f"cs{s}") for s in range(2)]

            def vocab_chunks(m, chunks):
                hp = hTr[m % 3]
                for n in chunks:
                    pv = pBv.tile([128, NV], F32, tag=f"v{n % 2}", name=f"v{n % 2}")
                    for k in range(8):
                        nc.tensor.matmul(
                            pv[:],
                            lhsT=hp[:, k // 4, k % 4, :, :],
                            rhs=CT16[:, n, k, :],
                            start=(k == 0),
                            stop=(k == 7),
                        )
                    pev = pB.tile([128, NV], F32, tag=f"pe{n % 2}", name=f"pe{n % 2}")
                    nc.vector.tensor_add(pev[:], pv[:], Cb16[:, ts(n, NV)])
                    nc.sync.dma_start(out_d[ts(m, 128), ts(n, NV)], pev[:])

            def step(t):
                parity = t % 2
                slot = (t // 2) % 3
                utile = ut[t % 4]
                vm = (t - 2) // 2 if t >= 2 else None
                voff = 4 * parity

                cn = cst[t % 2]
                cp = cst[1 - t % 2]
                if t == 0:
                    nc.scalar.activation(sig[:, :], utile[:, :1536], SIG)
                    nc.scalar.activation(th[:], utile[:, 1536:], TANH)
                    nc.vector.tensor_mul(cn[:], sig[:, 0:512], th[:])
                    nc.vector.tensor_mul(h16[:], sig[:, 1024:1536], cn[:])
                else:
                    hprev = hTr[((t - 1) // 2) % 3]
                    pparity = (t - 1) % 2
                    gp = [
                        pBg.tile([128, 512], F32, tag=f"gp{g}", name=f"gp{g}")
                        for g in range(4)
                    ]
                    # c~ first so tanh + i*c~ start early; then i, f, o
                    for g in (3, 0, 1, 2):
                        for k in range(8):
                            hk = hprev[:, k // 4, k % 4, pparity, :]
                            # two concurrent col-group MMs (psum partitions
                            # 0:64 / 64:128); the sim's zero-region group
                            # check is partition-base-unaware, skip it
                            nc.tensor.matmul(
                                gp[g][0:64, :],
                                lhsT=hk,
                                rhs=W_s[:, k, g * 1024 : g * 1024 + 512],
                                start=(k == 0),
                                stop=(k == 7),
                                skip_group_check=True,
                            )
                            nc.tensor.matmul(
                                gp[g][64:128, :],
                                lhsT=hk,
                                rhs=W_s[:, k, g * 1024 + 512 : (g + 1) * 1024],
                                start=(k == 0),
                                stop=(k == 7),
                                skip_group_check=True,
                            )
                        # gs = psum + u  (bf16 out)
                        nc.vector.tensor_add(
                            gs[:, ts(g, 512)], gp[g][:], utile[:, ts(g, 512)]
                        )
                        if g == 3:
                            nc.scalar.activation(th[:], gs[:, ts(g, 512)], TANH)
                        else:
                            nc.scalar.activation(
                                sig[:, ts(g, 512)], gs[:, ts(g, 512)], SIG
                            )
                        if g == 0:
                            # e = i * c~ on gpsimd, off the DVE critical path
                            nc.gpsimd.tensor_mul(tmp1[:], sig[:, 0:512], th[:])
                        elif g == 1:
                            # d = f * c_prev, then c_new = d + e
                            nc.vector.tensor_mul(cn[:], sig[:, 512:1024], cp[:])
                            nc.vector.tensor_add(cn[:], cn[:], tmp1[:])
                        elif g == 2:
                            nc.vector.tensor_mul(h16[:], sig[:, 1024:1536], cn[:])

                # prefetch u for step t+3 (after the gpsimd chain op is queued)
                if t + 3 < T:
                    gather_u(t + 3)

                # first half of this step's vocab tile while h resolves
                if vm is not None:
                    vocab_chunks(vm, range(voff, voff + 2))
                # transpose h into hT layout: 4x [128,128] PE transposes
                for cc in range(4):
                    tp = pBt.tile([128, 128], BF16, tag="htp")
                    nc.tensor.transpose(tp[:], h16[:, ts(cc, 128)], id128b[:])
                    nc.vector.tensor_copy(hTr[slot][:, :, cc, parity, :], tp[:])
                if vm is not None:
                    vocab_chunks(vm, range(voff + 2, voff + 4))

            step(0)
            load_weights()
            step(1)
            load_vocab_weights()
            for t in range(2, T):
                step(t)
            vocab_chunks(19, range(8))

    nc.compile()
    return nc


def kernel(**inputs):
    captions = np.asarray(inputs["captions"])
    B_w = np.asarray(inputs["B_w"], dtype=np.float32)
    V_w = np.asarray(inputs["V_w"], dtype=np.float32)
    V_b = np.asarray(inputs["V_b"], dtype=np.float32)
    S_w = np.asarray(inputs["S_w"], dtype=np.float32)
    S_b = np.asarray(inputs["S_b"], dtype=np.float32)
    U_w = np.asarray(inputs["U_w"], dtype=np.float32)
    U_b = np.asarray(inputs["U_b"], dtype=np.float32)
    W_w = np.asarray(inputs["W_w"], dtype=np.float32)
    W_b = np.asarray(inputs["W_b"], dtype=np.float32)
    C_w = np.asarray(inputs["C_w"], dtype=np.float32)
    C_b = np.asarray(inputs["C_b"], dtype=np.float32)

    bf16 = ml_dtypes.bfloat16

    # --- host-side weight prep ---
    # fused input path: BM = B_w @ (U S V)^T + gate_bias   [V, 4H]
    M2 = np.stack([U_w[g] @ S_w[g] @ V_w[g] for g in range(4)])  # [4, H, E]
    bs = np.einsum("gf,gof->go", V_b, S_w) + S_b
    bu = np.einsum("gf,ghf->gh", bs, U_w) + U_b
    gate_bias = bu + W_b  # [4, H]
    BM = B_w @ M2.reshape(4 * H, E).T + gate_bias.reshape(1, 4 * H)  # [V, 4H]
    # [v, half, g*512+j] = BM[v, g*1024+512*half+j]; halves interleaved row-wise
    BMab = np.ascontiguousarray(
        BM.reshape(V, 4, 2, 512).transpose(0, 2, 1, 3).reshape(2 * V, 2048).astype(bf16)
    )

    # W, gate-major cols, k-chunked [ki, ko, 4H]
    Wgm = W_w.transpose(2, 0, 1).reshape(H, 4 * H)
    Wk = np.ascontiguousarray(
        Wgm.reshape(8, 128, 4 * H).transpose(1, 0, 2).astype(bf16)
    )

    # captions doubled: partition p fetches table row 2*cap + (p // 64)
    capBT = captions.astype(np.int32)  # [B, T]
    idx = np.ascontiguousarray(
        np.concatenate([2 * capBT, 2 * capBT + 1], axis=0)
    )  # [128, T]

    CT = C_w.T  # [H, V]

    nc = _build()

    in_maps = []
    for c in range(NCORES):
        CTc = CT[:, c * VS : (c + 1) * VS]
        # [ki, n, ko, j] = CT[ko*128+ki, n*500+j] — chunk-contiguous slices
        CTr = np.ascontiguousarray(
            CTc.reshape(8, 128, 8, NV).transpose(1, 2, 0, 3).astype(bf16)
        )
        in_maps.append(
            {
                "idx": idx,
                "BMab": BMab,
                "Wg": Wk,
                "CTc": CTr,
                "Cb": np.ascontiguousarray(
                    np.broadcast_to(C_b[c * VS : (c + 1) * VS], (128, VS)).astype(bf16)
                ),
            }
        )

    global _last_in_maps
    _last_in_maps = in_maps

    res = run_bass_kernel_spmd(nc, in_maps, list(range(NCORES)))
    out = np.concatenate([res.results[c]["out"] for c in range(NCORES)], axis=1)
    return out.astype(np.float32)


_last_in_maps = None
